# revision 23
# baseline (speedup 1.0000x reference)
"""Trainium2 Bass kernel for fused attention block (B=2, S=2048, H=1024, N=16, D=64).

Sharding: 8 cores = 2 batches (DP) x 4 head-groups (TP, 4 heads each).

Structure per core:
  phase Q: QKV projection (PE) + LN + RoPE (DVE/ACT/gpsimd) + transposes.
    Projection groups are emitted one group ahead of the LN/RoPE work so the
    DVE queue keeps evacuating PSUM while PE streams matmuls.
  attention: 4 units = (q-half x head-pair). Within a unit the two heads'
    QK matmuls are row-packed (K=64 -> PE rows 0-63 / 64-127 via
    tile_position) and run concurrently; exp on ACT (the bottleneck engine)
    alternates heads so it stays saturated; PV keeps V' (with ones column)
    stationary. PSUM: scA+scB (4 banks) + pvA+pvB (4 banks).
  After each unit the normalized outputs ship via a small AllGather (4 per
  kernel) so the output projection only waits for the last quarter.
  V-bias is folded into the output bias on the host (exact).
"""

import numpy as np
import ml_dtypes

import concourse.bass as bass
from concourse import bacc
import concourse.mybir as mybir
import concourse.tile as tile
from concourse.masks import make_identity

# problem shape (hardcoded per contract)
B, S, H, NH, D = 2, 2048, 1024, 16, 64
EPS = 1.0 / 65530.0
NCORES = 8
HPC = 4            # heads per core
OC = HPC * D       # 256 head-dims per core
P = 128
SB = S // P        # 16 s-blocks
KC = H // P        # 8 contraction chunks of 128
D2 = D // 2
SCALE = 1.0 / 8.0  # 1/sqrt(D)
DV = D + 1         # V columns per head incl. ones column
SH = S // 2        # q-half size (1024)

BF = mybir.dt.bfloat16
F32 = mybir.dt.float32
ALU = mybir.AluOpType
ACTF = mybir.ActivationFunctionType


def build_nc():
    nc = bacc.Bacc(num_devices=NCORES)

    hT = nc.declare_dram_parameter("hT", [H, S], BF, isOutput=False)
    qwT = nc.declare_dram_parameter("qwT", [H, OC], BF, isOutput=False)
    kwT = nc.declare_dram_parameter("kwT", [H, OC], BF, isOutput=False)
    vwT = nc.declare_dram_parameter("vwT", [H, OC], BF, isOutput=False)
    owT = nc.declare_dram_parameter("owT", [H, OC], BF, isOutput=False)
    wsum = nc.declare_dram_parameter("wsum", [H, 2 * HPC], BF, isOutput=False)
    qkbs = nc.declare_dram_parameter("qkbs", [P, 2 * HPC], F32, isOutput=False)
    qb = nc.declare_dram_parameter("qb", [P, OC], F32, isOutput=False)
    kb = nc.declare_dram_parameter("kb", [P, OC], F32, isOutput=False)
    ob = nc.declare_dram_parameter("ob", [P, OC], F32, isOutput=False)
    cosd = nc.declare_dram_parameter("cosd", [S, D], F32, isOutput=False)
    sind = nc.declare_dram_parameter("sind", [S, D], F32, isOutput=False)
    out = nc.declare_dram_parameter("out", [S, OC], F32, isOutput=True)

    with tile.TileContext(nc) as tc:
        with tc.tile_pool(name="persist", bufs=1) as persist:
            # transposed q/k in head-pair chunks: chunk c rows 0..63 = head 2c,
            # rows 64..127 = head 2c+1
            qT2 = persist.tile([P, 2, S], BF)
            kT2 = persist.tile([P, 2, S], BF)
            # v in [s, head*(D+1)] layout: D data cols + 1 ones col per head
            Vp = persist.tile([P, SB, HPC * DV], BF)
            for h in range(HPC):
                nc.gpsimd.memset(Vp[:, :, h * DV + D : (h + 1) * DV], 1.0)
            attnT = persist.tile([D, HPC, S], BF)      # normalized [d, h, s]
            aT = persist.tile([P, 2, KC, SH], BF)      # gathered ctx [p, qh, kc, s]
            owT_sb = persist.tile([P, KC, OC], BF)
            ob_sb = persist.tile([P, OC], F32)
            cos_sb = persist.tile([P, SB, D], F32)
            sin_sb = persist.tile([P, SB, D], F32)
            cos4 = persist.tile([P, SB, HPC, D], BF)
            sin4 = persist.tile([P, SB, HPC, D], BF)
            qb_sb = persist.tile([P, OC], F32)
            kb_sb = persist.tile([P, OC], F32)

            muraw = persist.tile([P, SB, 2 * HPC], F32)
            qkbs_sb = persist.tile([P, 2 * HPC], F32)
            mu_q = persist.tile([P, SB, HPC], F32)
            mu_k = persist.tile([P, SB, HPC], F32)
            var_q = persist.tile([P, SB, HPC], F32)
            var_k = persist.tile([P, SB, HPC], F32)
            rstd_q = persist.tile([P, SB, HPC], F32)
            rstd_k = persist.tile([P, SB, HPC], F32)
            std_q = persist.tile([P, SB, HPC], F32)
            std_k = persist.tile([P, SB, HPC], F32)
            eps_t = persist.tile([P, 1], F32)
            nc.gpsimd.memset(eps_t[:], EPS)

            GS = 8                    # s-blocks per prologue pipeline group
            NG = SB // GS

            def stats_grp(xf, g, mu, var, pool, tagp):
                # mean comes from the projection's column-sum outputs (muraw);
                # only the x^2 reduction runs on DVE here
                gs = slice(g * GS, (g + 1) * GS)
                moff = 0 if tagp == "q" else HPC
                nc.vector.tensor_scalar_mul(mu[:, gs], muraw[:, gs, moff : moff + HPC], 1.0 / D)
                sqf = pool.tile([P, GS, OC], F32, name=f"sqf{tagp}{g}", tag="sqf", bufs=1)
                nc.scalar.activation(sqf[:], xf[:, gs], ACTF.Square)
                sv = sqf[:].rearrange("p s (h d) -> p s h d", h=HPC)
                nc.vector.tensor_reduce(out=var[:, gs], in_=sv, axis=mybir.AxisListType.X, op=ALU.add)
                nc.vector.tensor_scalar_mul(var[:, gs], var[:, gs], 1.0 / D)
                mu2 = pool.tile([P, GS, HPC], F32, name=f"mu2{tagp}{g}", tag="mu2", bufs=1)
                nc.vector.tensor_tensor(out=mu2[:], in0=mu[:, gs], in1=mu[:, gs], op=ALU.mult)
                nc.vector.tensor_tensor(out=var[:, gs], in0=var[:, gs], in1=mu2[:], op=ALU.subtract)

            def ln_rope_transpose_grp(xf, g, mu, rstd, xT2, pool, eng, tagp, pe_transpose=None, tt=None):
                gs = slice(g * GS, (g + 1) * GS)
                xv = xf[:, gs].rearrange("p s (h d) -> p s h d", h=HPC)
                mu_b = mu[:, gs, :, None].to_broadcast((P, GS, HPC, D))
                rs_b = rstd[:, gs, :, None].to_broadcast((P, GS, HPC, D))
                tt.tensor_tensor(out=xv, in0=xv, in1=mu_b, op=ALU.subtract)
                tt.tensor_tensor(out=xv, in0=xv, in1=rs_b, op=ALU.mult)
                cb = cos4[:, gs]
                s1 = sin4[:, gs, :, 0:D2]
                s2 = sin4[:, gs, :, D2:D]
                ca = pool.tile([P, GS, HPC, D], BF, name=f"ca{tagp}{g}", tag="ca", bufs=1)
                th = pool.tile([P, GS, HPC, D2], BF, name=f"th{tagp}{g}", tag="th", bufs=1)
                t2 = pool.tile([P, GS, HPC, D2], BF, name=f"t2{tagp}{g}", tag="t2", bufs=1)
                rx = pool.tile([P, GS, HPC, D], BF, name=f"rx{tagp}{g}", tag="rx", bufs=1)
                tt.tensor_tensor(out=th[:], in0=xv[:, :, :, D2:D], in1=s1, op=ALU.mult)
                tt.tensor_tensor(out=t2[:], in0=xv[:, :, :, 0:D2], in1=s2, op=ALU.mult)
                tt.tensor_tensor(out=ca[:], in0=xv, in1=cb, op=ALU.mult)
                tt.tensor_tensor(out=rx[:, :, :, 0:D2], in0=ca[:, :, :, 0:D2], in1=th[:], op=ALU.subtract)
                tt.tensor_tensor(out=rx[:, :, :, D2:D], in0=ca[:, :, :, D2:D], in1=t2[:], op=ALU.add)
                rx2 = rx[:].rearrange("p s h d -> p s (h d)")
                for c in range(2):
                    for si in range(GS):
                        sb = g * GS + si
                        if pe_transpose is not None:
                            tpool, ident = pe_transpose
                            pst = tpool.tile([P, P], BF, name=f"pst{tagp}{c}{sb}", tag="pst")
                            nc.tensor.transpose(pst[:], rx2[:, si, c * P : (c + 1) * P], ident[:])
                            nc.scalar.activation(xT2[:, c, sb * P : (sb + 1) * P], pst[:], ACTF.Copy)
                        else:
                            eng.dma_start(
                                xT2[:, c, sb * P : (sb + 1) * P],
                                rx2[:, si, c * P : (c + 1) * P],
                                transpose=True,
                            )

            # ---------------- phase Q ----------------------------------
            with tc.tile_pool(name="pw", bufs=1) as pw, \
                 tc.tile_pool(name="projpsum", bufs=3, space="PSUM") as projpsum, \
                 tc.tile_pool(name="tpsum", bufs=2, space="PSUM") as tpsum, \
                 tc.tile_pool(name="ptmp", bufs=3) as ptmp:
                ident = pw.tile([P, P], BF)
                make_identity(nc, ident)
                # PE warm-up: sustained matmul burst releases the HAM clock
                # throttle (needs ~3.4us of continuous PE activity)
                junk = pw.tile([P, 512], BF)
                nc.gpsimd.memset(junk[:], 1.0)
                wps = projpsum.tile([P, 3 * OC + 2 * HPC], F32, name="wps", tag="pq")
                for _ in range(20):
                    nc.tensor.matmul(wps[:, 0:512], ident[:], junk[:], start=True, stop=True)

                # DMAs ordered so the first projection group unblocks ASAP
                KH = KC // 2
                hT_sb = pw.tile([P, KC, S], BF)
                hT_r = hT[:].rearrange("(a p) s -> p a s", p=P)
                qkvwT_sb = pw.tile([P, KC, 3 * OC + 2 * HPC], BF)
                qw_r = qwT[:].rearrange("(a p) o -> p a o", p=P)
                kw_r = kwT[:].rearrange("(a p) o -> p a o", p=P)
                vw_r = vwT[:].rearrange("(a p) o -> p a o", p=P)
                ws_r = wsum[:].rearrange("(a p) o -> p a o", p=P)
                nc.sync.dma_start(hT_sb[:, 0:KH], hT_r[:, 0:KH])
                nc.scalar.dma_start(qkvwT_sb[:, 0:KH, 0:OC], qw_r[:, 0:KH])
                nc.scalar.dma_start(qkvwT_sb[:, 0:KH, OC : 2 * OC], kw_r[:, 0:KH])
                nc.scalar.dma_start(qkvwT_sb[:, 0:KH, 2 * OC : 3 * OC], vw_r[:, 0:KH])
                nc.scalar.dma_start(qkvwT_sb[:, 0:KH, 3 * OC :], ws_r[:, 0:KH])
                nc.sync.dma_start(qb_sb[:], qb[:])
                nc.sync.dma_start(kb_sb[:], kb[:])
                nc.sync.dma_start(qkbs_sb[:], qkbs[:])
                nc.sync.dma_start(cos_sb[:], cosd[:].rearrange("(a p) d -> p a d", p=P))
                nc.sync.dma_start(sin_sb[:], sind[:].rearrange("(a p) d -> p a d", p=P))
                nc.scalar.activation(cos4[:], cos_sb[:, :, None, :].to_broadcast((P, SB, HPC, D)), ACTF.Copy)
                nc.scalar.activation(sin4[:], sin_sb[:, :, None, :].to_broadcast((P, SB, HPC, D)), ACTF.Copy)
                nc.sync.dma_start(hT_sb[:, KH:KC], hT_r[:, KH:KC])
                nc.scalar.dma_start(qkvwT_sb[:, KH:KC, 0:OC], qw_r[:, KH:KC])
                nc.scalar.dma_start(qkvwT_sb[:, KH:KC, OC : 2 * OC], kw_r[:, KH:KC])
                nc.scalar.dma_start(qkvwT_sb[:, KH:KC, 2 * OC : 3 * OC], vw_r[:, KH:KC])
                nc.scalar.dma_start(qkvwT_sb[:, KH:KC, 3 * OC :], ws_r[:, KH:KC])
                nc.gpsimd.dma_start(ob_sb[:], ob[:])
                nc.gpsimd.dma_start(owT_sb[:], owT[:].rearrange("(a p) o -> p a o", p=P))

                qf = pw.tile([P, SB, OC], F32)
                kf = pw.tile([P, SB, OC], F32)

                def proj_grp(g):
                    for si in range(GS):
                        sb = g * GS + si
                        pq = projpsum.tile([P, 3 * OC + 2 * HPC], F32, name=f"pq{sb}", tag="pq")
                        for kc in range(KC):
                            lhsp = hT_sb[:, kc, sb * P : (sb + 1) * P]
                            nc.tensor.matmul(
                                pq[:, 0:512], lhsp, qkvwT_sb[:, kc, 0:512],
                                start=(kc == 0), stop=(kc == KC - 1),
                            )
                            nc.tensor.matmul(
                                pq[:, 512:], lhsp, qkvwT_sb[:, kc, 512:],
                                start=(kc == 0), stop=(kc == KC - 1),
                            )
                        nc.vector.tensor_tensor(out=muraw[:, sb], in0=pq[:, 3 * OC :], in1=qkbs_sb[:], op=ALU.add)
                        nc.vector.tensor_tensor(out=qf[:, sb], in0=pq[:, 0:OC], in1=qb_sb[:], op=ALU.add)
                        nc.vector.tensor_tensor(out=kf[:, sb], in0=pq[:, OC : 2 * OC], in1=kb_sb[:], op=ALU.add)
                        nc.scalar.activation(
                            Vp[:, sb].rearrange("p (h e) -> p h e", h=HPC)[:, :, 0:D],
                            pq[:, 2 * OC : 3 * OC].rearrange("p (h d) -> p h d", h=HPC),
                            ACTF.Copy,
                        )

                # All projection matmuls first (dense PE stream, warms the HAM),
                # then q LN/RoPE (PE transposes), then k LN/RoPE (DMA
                # transposes) — the late k group overlaps the attention start.
                for g in range(NG):
                    proj_grp(g)
                for g in range(NG):
                    gsl = slice(g * GS, (g + 1) * GS)
                    stats_grp(kf, g, mu_k, var_k, ptmp, "k")
                    nc.scalar.activation(std_k[:, gsl], var_k[:, gsl], ACTF.Sqrt, bias=eps_t[:])
                    nc.vector.reciprocal(rstd_k[:, gsl], std_k[:, gsl])
                for g in range(NG):
                    gsl = slice(g * GS, (g + 1) * GS)
                    stats_grp(qf, g, mu_q, var_q, ptmp, "q")
                    nc.scalar.activation(std_q[:, gsl], var_q[:, gsl], ACTF.Sqrt, bias=eps_t[:])
                    nc.vector.reciprocal(rstd_q[:, gsl], std_q[:, gsl])
                    nc.vector.tensor_scalar_mul(rstd_q[:, gsl], rstd_q[:, gsl], SCALE)
                    ln_rope_transpose_grp(qf, g, mu_q, rstd_q, qT2, ptmp, nc.scalar, "q",
                                          pe_transpose=(tpsum, ident), tt=nc.vector)
                for g in range(NG):
                    ln_rope_transpose_grp(kf, g, mu_k, rstd_k, kT2, ptmp, nc.sync, "k",
                                          tt=nc.gpsimd)

            # ---------------- phase A: attention ------------------------
            with tc.tile_pool(name="dram", bufs=1, space="DRAM") as dram:
                cc_in0 = dram.tile([P, S], BF)     # pair 0, both q-halves
                cc_out0 = dram.tile([4 * P, S], BF)

                with tc.tile_pool(name="probs", bufs=3) as probspool, \
                     tc.tile_pool(name="apsum", bufs=1, space="PSUM") as apsum, \
                     tc.tile_pool(name="atmp", bufs=2) as atmp:

                    def qk_packed(hp, qh, t, scA, scB):
                        tsl = slice(t * P, (t + 1) * P)
                        for q4 in range(2):
                            nsl = slice(q4 * 512, (q4 + 1) * 512)
                            qsl = slice(qh * SH + q4 * 512, qh * SH + (q4 + 1) * 512)
                            nc.tensor.matmul(
                                scA[:, nsl], kT2[0:D, hp, tsl], qT2[0:D, hp, qsl],
                                start=True, stop=True,
                            )
                            nc.tensor.matmul(
                                scB[:, nsl], kT2[D:P, hp, tsl], qT2[D:P, hp, qsl],
                                start=True, stop=True,
                            )

                    def pv(h, t, pvp, probs_t, nspl):
                        for n0, nw in nspl:
                            nsl = slice(n0, n0 + nw)
                            nc.tensor.matmul(
                                pvp[:, nsl],
                                Vp[:, t, h * DV : (h + 1) * DV],
                                probs_t[:, nsl],
                                start=(t == 0), stop=(t == SB - 1),
                            )

                    def normalize(h, q0, w, pvp, u, act_recip=False):
                        # Evacuate PSUM fast (frees the pv banks for the next
                        # unit), then attnT = pvf[0:D] / pvf[D] off-psum.  The
                        # reciprocal runs on DVE (iterative, slow, fine off the
                        # critical path) or via ACT ln/exp for the tail unit.
                        qsl = slice(q0, q0 + w)
                        pvf = atmp.tile([DV, w], F32, name=f"pvf{u}{h}", tag="pvf", bufs=2)
                        nc.vector.tensor_copy(out=pvf[:], in_=pvp[:])
                        rb = atmp.tile([D, w], F32, name=f"rb{u}{h}", tag="rb", bufs=2)
                        if act_recip:
                            lnr = atmp.tile([1, w], F32, name=f"lnr{u}{h}", tag="lnr", bufs=2)
                            nc.scalar.activation(lnr[:], pvf[D : D + 1, :], ACTF.Ln)
                            rinv = atmp.tile([1, w], F32, name=f"rinv{u}{h}", tag="rinv", bufs=2)
                            nc.scalar.activation(rinv[:], lnr[:], ACTF.Exp, scale=-1.0)
                            nc.scalar.dma_start(rb[:], rinv[0:1, None, :].to_broadcast((1, D, w)))
                        else:
                            nc.scalar.dma_start(rb[:], pvf[D : D + 1, None, :].to_broadcast((1, D, w)))
                            nc.vector.reciprocal(rb[:], rb[:])
                        nc.vector.tensor_tensor(
                            out=attnT[:, h, qsl], in0=pvf[0:D, :], in1=rb[:], op=ALU.mult,
                        )

                    # units: pair 0 in q-halves; pair 1 in q-chunks of
                    # 1024/768/256 so the final AllGather is tiny
                    UNITS = [(0, 0, 1024), (0, 1024, 1024),
                             (1, 0, 1024), (1, 1024, 768), (1, 1792, 256)]
                    for u, (hp, q0, w) in enumerate(UNITS):
                        hA, hB = 2 * hp, 2 * hp + 1
                        nspl = [(i, min(512, w - i)) for i in range(0, w, 512)]
                        pvA = apsum.tile([DV, w], F32, name=f"pvA{u}", tag="pvA")
                        pvB = apsum.tile([DV, w], F32, name=f"pvB{u}", tag="pvB")
                        probsA, probsB = {}, {}
                        for t in range(SB):
                            scA = apsum.tile([P, w], F32, name=f"scA{u}_{t}", tag="scA")
                            scB = apsum.tile([P, w], F32, name=f"scB{u}_{t}", tag="scB")
                            tsl = slice(t * P, (t + 1) * P)
                            for n0, nw in nspl:
                                nsl = slice(n0, n0 + nw)
                                qsl = slice(q0 + n0, q0 + n0 + nw)
                                nc.tensor.matmul(
                                    scA[:, nsl], kT2[0:D, hp, tsl], qT2[0:D, hp, qsl],
                                    start=True, stop=True,
                                )
                                nc.tensor.matmul(
                                    scB[:, nsl], kT2[D:P, hp, tsl], qT2[D:P, hp, qsl],
                                    start=True, stop=True,
                                )
                            probsA[t] = probspool.tile([P, w], BF, name=f"pA{u}_{t}", tag="pA")
                            probsB[t] = probspool.tile([P, w], BF, name=f"pB{u}_{t}", tag="pB")
                            nc.scalar.activation(probsA[t][:], scA[:], ACTF.Exp)
                            nc.scalar.activation(probsB[t][:], scB[:], ACTF.Exp)
                            if t >= 1:
                                pv(hA, t - 1, pvA, probsA.pop(t - 1), nspl)
                                pv(hB, t - 1, pvB, probsB.pop(t - 1), nspl)
                        pv(hA, SB - 1, pvA, probsA.pop(SB - 1), nspl)
                        pv(hB, SB - 1, pvB, probsB.pop(SB - 1), nspl)
                        normalize(hA, q0, w, pvA, u, act_recip=True)
                        normalize(hB, q0, w, pvB, u, act_recip=True)
                        if u == 0:
                            continue
                        if u == 1:
                            # pair 0 complete: ship both q-halves at once
                            nc.gpsimd.dma_start(
                                cc_in0[:].rearrange("(hh p) s -> p hh s", p=D),
                                attnT[:, 0:2, :],
                            )
                            nc.gpsimd.collective_compute(
                                "AllGather", ALU.bypass,
                                replica_groups=[[0, 1, 2, 3], [4, 5, 6, 7]],
                                ins=[cc_in0[:].opt()], outs=[cc_out0[:].opt()],
                            )
                            co = cc_out0[:].rearrange("(g p) (q s) -> p q g s", p=P, q=2)
                            nc.gpsimd.dma_start(
                                aT[:].rearrange("p q (g j) s -> p j q g s", j=2)[:, 0],
                                co,
                            )
                        else:
                            cc_i = dram.tile([P, w], BF, name=f"cci{u}")
                            cc_o = dram.tile([4 * P, w], BF, name=f"cco{u}")
                            nc.gpsimd.dma_start(
                                cc_i[:].rearrange("(hh p) s -> p hh s", p=D),
                                attnT[:, 2:4, q0 : q0 + w],
                            )
                            nc.gpsimd.collective_compute(
                                "AllGather", ALU.bypass,
                                replica_groups=[[0, 1, 2, 3], [4, 5, 6, 7]],
                                ins=[cc_i[:].opt()], outs=[cc_o[:].opt()],
                            )
                            qhx, c0 = q0 // SH, q0 % SH
                            nc.gpsimd.dma_start(
                                aT[:, qhx].rearrange("p (g j) s -> p j g s", j=2)[:, 1, :, c0 : c0 + w],
                                cc_o[:].rearrange("(g p) s -> p g s", p=P),
                            )

            # ---------------- phase O: output projection ------------
                with tc.tile_pool(name="opsum", bufs=4, space="PSUM") as opsum, \
                     tc.tile_pool(name="otmp", bufs=3) as otmp:
                    for sb in range(SB):
                        qh, sx = sb // 8, sb % 8
                        pso = opsum.tile([P, OC], F32, name=f"pso{sb}", tag="pso")
                        for kc in range(KC):
                            nc.tensor.matmul(
                                pso[:],
                                aT[:, qh, kc, sx * P : (sx + 1) * P],
                                owT_sb[:, kc],
                                start=(kc == 0), stop=(kc == KC - 1),
                            )
                        of = otmp.tile([P, OC], F32, name=f"of{sb}", tag="of")
                        nc.vector.tensor_tensor(out=of[:], in0=pso[:], in1=ob_sb[:], op=ALU.add)
                        nc.sync.dma_start(out[sb * P : (sb + 1) * P, :], of[:])

    nc.finalize()
    return nc


_NC_CACHE = None


def _get_nc():
    global _NC_CACHE
    if _NC_CACHE is None:
        _NC_CACHE = build_nc()
    return _NC_CACHE


def _prep_in_maps(inputs):
    bf16 = ml_dtypes.bfloat16
    hidden = np.asarray(inputs["hidden_states"], np.float32)
    cos = np.ascontiguousarray(np.asarray(inputs["cos"], np.float32))
    sin = np.ascontiguousarray(np.asarray(inputs["sin"], np.float32))
    q_w = np.asarray(inputs["q_w"], np.float32)
    q_b = np.asarray(inputs["q_b"], np.float32)
    kv_w = np.asarray(inputs["kv_w"], np.float32)
    kv_b = np.asarray(inputs["kv_b"], np.float32)
    o_w = np.asarray(inputs["o_w"], np.float32)
    o_b = np.asarray(inputs["o_b"], np.float32)

    hT = [np.ascontiguousarray(hidden[b].T).astype(bf16) for b in range(B)]
    v_b = kv_b[H:]
    # V-bias folds into the output bias: softmax(..)@(v+vb) @ o_w.T
    #   = softmax(..)@v @ o_w.T + vb @ o_w.T
    ob_eff = o_b + o_w @ v_b

    in_maps = []
    for c in range(NCORES):
        b, hg = divmod(c, 4)
        sl = slice(hg * OC, (hg + 1) * OC)
        vsl = slice(H + hg * OC, H + (hg + 1) * OC)
        qws = q_w[sl].reshape(HPC, D, H).sum(1).T          # [H, 4]
        kws = kv_w[sl].reshape(HPC, D, H).sum(1).T         # [H, 4]
        wsum = np.ascontiguousarray(np.concatenate([qws, kws], 1)).astype(bf16)
        qbs = q_b[sl].reshape(HPC, D).sum(1)               # [4]
        kbs = kv_b[sl].reshape(HPC, D).sum(1)              # [4]
        qkbs = np.ascontiguousarray(np.broadcast_to(
            np.concatenate([qbs, kbs]), (P, 2 * HPC))).astype(np.float32)
        in_maps.append({
            "hT": hT[b],
            "wsum": wsum,
            "qkbs": qkbs,
            "qwT": np.ascontiguousarray(q_w[sl].T).astype(bf16),
            "kwT": np.ascontiguousarray(kv_w[sl].T).astype(bf16),
            "vwT": np.ascontiguousarray(kv_w[vsl].T).astype(bf16),
            "owT": np.ascontiguousarray(o_w[sl].T).astype(bf16),
            "qb": np.ascontiguousarray(np.broadcast_to(q_b[sl], (P, OC))),
            "kb": np.ascontiguousarray(np.broadcast_to(kv_b[sl], (P, OC))),
            "ob": np.ascontiguousarray(np.broadcast_to(ob_eff[sl], (P, OC))),
            "cosd": cos,
            "sind": sin,
        })
    return in_maps


def _assemble(results):
    out = np.empty((B, S, H), np.float32)
    for c in range(NCORES):
        b, hg = divmod(c, 4)
        out[b, :, hg * OC : (hg + 1) * OC] = results[c]["out"]
    return out


def _enable_ldw_opt():
    try:
        from concourse.compiler_utils import get_compiler_flags, set_compiler_flags
        flags = get_compiler_flags()
        patched = [f.replace("--enable-ldw-opt=false", "--enable-ldw-opt=true") for f in flags]
        if patched != flags:
            set_compiler_flags(patched)
    except Exception:
        pass


def kernel(**inputs):
    from concourse.bass_utils import run_bass_kernel_spmd

    _enable_ldw_opt()

    nc = _get_nc()
    in_maps = _prep_in_maps(inputs)
    res = run_bass_kernel_spmd(nc, in_maps, list(range(NCORES)))
    results = res.results if hasattr(res, "results") else res
    return _assemble(results)
